# revision 26
# baseline (speedup 1.0000x reference)
"""Causal self-attention (K/Q swapped variant) on 8 trn2 NeuronCores.

Sharding: core c = (b, g) with b = c // 4 (batch), g = c % 4 (head group of
4 heads).  Each core computes, for its batch and heads, the full attention
and a partial output projection (its heads' rows of Wproj); the host sums
the 4 bf16 partials per batch (fp32 accumulate) and adds bproj.

Per-core kernel (bf16 matmuls, fp32 PSUM accumulation):
  - x[b]^T arrives pre-transposed (bf16) from host as [D, N].
  - K^T, Q^T, V^T per head-pair: [128, 512] PSUM tiles (2 heads stacked on
    partitions) via W-stationary matmuls; k/q/v biases are folded in as a
    K=1 seed matmul (bias ⊗ ones) that starts each accumulation group.
  - V^T is PE-transposed into per-head [m, 65] tiles whose 65th column is
    1.0 (softmax row-sums fall out of the O matmul for free).
  - Scores for both heads of a pair go into one [128, 1024] PSUM quad
    (hh0 cols 0:512, hh1 cols 512:1024, different banks so the two K=64
    row-tiled matmuls run concurrently); ONE ACT exp per quad (scale=1/8),
    with the live-column trim applied exactly on diagonal quads.
  - Causal masking: one GPSIMD tensor-tensor multiply per diagonal quad
    over the two 128-wide on-diagonal blocks (3D access pattern); fully
    masked columns are never computed or read.
  - O_aug = V_aug^T . E accumulated over m-blocks: rows 0-63 unnormalized
    output^T, row 64 the softmax denominator.
  - normalize: reciprocal_approx_fast of row 64 straight off PSUM, PE
    outer-product broadcast to 64 partitions, one DVE multiply fusing the
    bf16 cast into otp.
  - partial out (bf16) = sum_h O_h^T.T @ Wproj[head rows] in PSUM, emitted
    per n-block so the projection fills PE gaps during the next block's
    attention.

PSUM budget (8 banks): scores quads 2x[128,1024] = 4, O accumulators
2x[65,512] = 2 (pairs processed serially), shared kqv/transpose/bc/proj
pool 2x[128,512] = 2.
"""

import os
import sys

if "/opt/trn_rl_repo" not in sys.path:
    sys.path.insert(0, "/opt/trn_rl_repo")

import numpy as np

B, N, D, H = 2, 2048, 1024, 16
DK = 64
NCORES = 8
GROUPS = 4          # head groups
HPC = H // GROUPS   # 4 heads per core
CH = D // 128       # 8 contraction chunks
NB = N // 512       # 4 n-blocks
MBS = N // 128      # 16 m-blocks

_CACHE = {}


def _build_program():
    import concourse.bacc as bacc
    import concourse.mybir as mybir
    from concourse.tile import TileContext
    from contextlib import ExitStack

    f32 = mybir.dt.float32
    bf = mybir.dt.bfloat16
    EXP = mybir.ActivationFunctionType.Exp

    nc = bacc.Bacc(
        "TRN2",
        target_bir_lowering=False,
        debug=False,
        enable_asserts=False,
        num_devices=NCORES,
    )

    xT = nc.dram_tensor("xT", [D, N], bf, kind="ExternalInput").ap()
    wk = nc.dram_tensor("wk", [128, CH * 256], bf, kind="ExternalInput").ap()
    wq = nc.dram_tensor("wq", [128, CH * 256], bf, kind="ExternalInput").ap()
    wv = nc.dram_tensor("wv", [128, CH * 256], bf, kind="ExternalInput").ap()
    wp = nc.dram_tensor("wp", [128, 2 * D], bf, kind="ExternalInput").ap()
    biasrow = nc.dram_tensor("biasrow", [1, 768], bf, kind="ExternalInput").ap()
    onesrow = nc.dram_tensor("onesrow", [1, 512], bf, kind="ExternalInput").ap()
    mask2 = nc.dram_tensor("mask2", [128, 256], bf, kind="ExternalInput").ap()
    ident = nc.dram_tensor("ident", [128, 128], bf, kind="ExternalInput").ap()
    ones2d = nc.dram_tensor("ones2d", [128, 16], bf, kind="ExternalInput").ap()
    onesf32 = nc.dram_tensor("onesf32", [1, 64], f32, kind="ExternalInput").ap()
    out_p = nc.dram_tensor("out_p", [N, D], bf, kind="ExternalOutput").ap()

    with TileContext(nc) as tc, ExitStack() as ctx:
        constp = ctx.enter_context(tc.tile_pool(name="const", bufs=1))
        storep = ctx.enter_context(tc.tile_pool(name="store", bufs=1))
        xtp = ctx.enter_context(tc.tile_pool(name="xt", bufs=16))
        vtp = ctx.enter_context(tc.tile_pool(name="vt", bufs=2))
        ep = ctx.enter_context(tc.tile_pool(name="e", bufs=6))
        rcp = ctx.enter_context(tc.tile_pool(name="rc", bufs=2))
        bcsp = ctx.enter_context(tc.tile_pool(name="bcs", bufs=3))
        oddp = ctx.enter_context(tc.tile_pool(name="odd", bufs=2))
        osp = ctx.enter_context(tc.tile_pool(name="os", bufs=3))
        sqp = ctx.enter_context(tc.tile_pool(name="sq", bufs=2, space="PSUM"))
        op_ = ctx.enter_context(tc.tile_pool(name="o", bufs=2, space="PSUM"))
        kvp = ctx.enter_context(tc.tile_pool(name="kv", bufs=2, space="PSUM"))

        # ---- constants / weights / x in SBUF ----
        # DMA traffic is split across the two HWDGE queues (sync, scalar);
        # 2-4KB per-partition lines keep packets big.  All of x loads up
        # front so kqv projections are always available as PE filler.
        wk_sb = constp.tile([128, CH * 256], bf, tag="wk")
        wq_sb = constp.tile([128, CH * 256], bf, tag="wq")
        wv_sb = constp.tile([128, CH * 256], bf, tag="wv")
        wp_sb = constp.tile([128, 2 * D], bf, tag="wp")
        biasrow_sb = constp.tile([1, 768], bf, tag="biasrow")
        onesrow_sb = constp.tile([1, 512], bf, tag="onesrow")
        mask2_sb = constp.tile([128, 256], bf, tag="mask2")
        ident_sb = constp.tile([128, 128], bf, tag="ident")
        ones2d_sb = constp.tile([128, 16], bf, tag="ones2d")
        # row 64 = ones; lives at partition 64 so the bc matmul's K=1
        # stationary and its rc moving operand share a base partition
        onesf32_sb = constp.tile([65, 64], f32, tag="onesf32")
        # rows 64-65: [1,0...;0,1...] selector for the per-pair bc matmul
        nc.sync.dma_start(biasrow_sb[:], biasrow[:, :])
        nc.sync.dma_start(onesrow_sb[:], onesrow[:, :])
        nc.sync.dma_start(ident_sb[:], ident[:, :])
        nc.sync.dma_start(mask2_sb[:], mask2[:, :])
        nc.sync.dma_start(ones2d_sb[:], ones2d[:, :])
        nc.sync.dma_start(onesf32_sb[64:65, :], onesf32[:, :])
        nc.sync.dma_start(wk_sb[:], wk[:, :])

        # x^T in two [128, 1024] halves per chunk, even chunks on the sync
        # queue, odd on the scalar queue
        xt = [[None] * CH for _ in range(2)]
        for half in range(2):
            for c in range(CH):
                t = xtp.tile([128, 1024], bf, tag="xt", name=f"xt{half}{c}")
                eng = nc.sync if c % 2 == 0 else nc.scalar
                eng.dma_start(
                    t[:],
                    xT[c * 128:(c + 1) * 128, half * 1024:(half + 1) * 1024],
                )
                xt[half][c] = t
            if half == 0:
                nc.scalar.dma_start(wq_sb[:], wq[:, :])
                nc.sync.dma_start(wv_sb[:], wv[:, :])
        nc.scalar.dma_start(wp_sb[:], wp[:, :])

        # ---- persistent activation storage ----
        kt = storep.tile([128, 2 * N], bf, tag="kt")    # [pairfeat, pair*N + n]
        qt = storep.tile([128, 2 * N], bf, tag="qt")
        v_sb = [storep.tile([128, MBS * 65], bf, tag=f"v{h}", name=f"v{h}")
                for h in range(HPC)]
        otp = [storep.tile([128, N], bf, tag=f"otp{p}", name=f"otp{p}")
               for p in range(2)]
        for h in range(HPC):
            nc.vector.tensor_copy(
                v_sb[h].rearrange("p (m c) -> p m c", c=65)[:, :, 64],
                ones2d_sb[:, 0:16],
            )

        mask3 = mask2_sb.rearrange("p (h c) -> p h c", h=2)

        for j in range(NB):
            nb = j
            xnb = [xt[nb // 2][c][:, (nb % 2) * 512:(nb % 2 + 1) * 512]
                   for c in range(CH)]

            # ---- K^T, Q^T, V^T projections for this n-block ----
            # bias folded in as a K=1 seed matmul starting each group
            for pair in range(2):
                for ti, (wsb, dst) in enumerate(((wk_sb, kt), (wq_sb, qt))):
                    ps = kvp.tile([128, 512], f32, tag="kv", name="kqv")
                    bcol = ti * 256 + pair * 128
                    nc.tensor.matmul(
                        ps[:], biasrow_sb[0:1, bcol:bcol + 128],
                        onesrow_sb[0:1, :], start=True, stop=False,
                    )
                    for c in range(CH):
                        nc.tensor.matmul(
                            ps[:],
                            wsb[:, c * 256 + pair * 128: c * 256 + (pair + 1) * 128],
                            xnb[c],
                            start=False,
                            stop=(c == CH - 1),
                        )
                    nc.vector.tensor_copy(
                        dst[:, pair * N + nb * 512: pair * N + (nb + 1) * 512],
                        ps[:],
                    )
                ps = kvp.tile([128, 512], f32, tag="kv", name="kqv")
                bcol = 2 * 256 + pair * 128
                nc.tensor.matmul(
                    ps[:], biasrow_sb[0:1, bcol:bcol + 128],
                    onesrow_sb[0:1, :], start=True, stop=False,
                )
                for c in range(CH):
                    nc.tensor.matmul(
                        ps[:],
                        wv_sb[:, c * 256 + pair * 128: c * 256 + (pair + 1) * 128],
                        xnb[c],
                        start=False,
                        stop=(c == CH - 1),
                    )
                vt = vtp.tile([128, 512], bf, tag="vt")
                nc.vector.tensor_copy(vt[:], ps[:])
                # transpose V^T -> per-head [m, 65] tiles
                for sub in range(4):
                    mb = nb * 4 + sub
                    pst = kvp.tile([128, 128], bf, tag="kv", name="pst")
                    nc.tensor.transpose(
                        pst[:], vt[:, sub * 128:(sub + 1) * 128], ident_sb[:]
                    )
                    nc.vector.tensor_copy(
                        v_sb[2 * pair][:, mb * 65: mb * 65 + 64], pst[:, 0:64]
                    )
                    nc.vector.tensor_copy(
                        v_sb[2 * pair + 1][:, mb * 65: mb * 65 + 64], pst[:, 64:128]
                    )

            # ---- attention for n-block j (needs m-blocks <= 4j+3) ----
            # pairs processed serially (o pool has 2 bufs = 1 pair)
            nm = 4 * j + 4
            for pair in range(2):
                o_ps = {}
                for hh in range(2):
                    o_ps[hh] = op_.tile([65, 512], f32, tag="o",
                                        name=f"o{j}{pair}{hh}")
                for mb in range(nm):
                    rdiag = mb - 4 * j
                    c0 = 128 * rdiag if rdiag > 0 else 0
                    # scores quad: both heads, different PSUM banks, the
                    # two K=64 matmuls run row-tiled concurrently
                    sq = sqp.tile([128, 1024], f32, tag="sq")
                    for hh in range(2):
                        base = hh * 64
                        nc.tensor.matmul(
                            sq[:, hh * 512 + c0: (hh + 1) * 512],
                            qt[base:base + 64,
                               pair * N + mb * 128: pair * N + (mb + 1) * 128],
                            kt[base:base + 64,
                               pair * N + j * 512 + c0: pair * N + (j + 1) * 512],
                        )
                    # ONE exp per quad, exact live-column trim
                    e = ep.tile([128, 1024], bf, tag="e")
                    sq3 = sq.rearrange("p (h c) -> p h c", h=2)
                    e3 = e.rearrange("p (h c) -> p h c", h=2)
                    nc.scalar.activation(e3[:, :, c0:], sq3[:, :, c0:], EXP,
                                         scale=0.125)
                    if rdiag >= 0:
                        # zero the two on-diagonal 128-blocks (GPSIMD; the
                        # fully-masked cols [0:c0) are never read)
                        blk = slice(rdiag * 128, (rdiag + 1) * 128)
                        nc.vector.tensor_mul(
                            e3[:, :, blk], e3[:, :, blk], mask3[:, :, 0:128]
                        )
                    for hh in range(2):
                        h = 2 * pair + hh
                        nc.tensor.matmul(
                            o_ps[hh][:, c0:512],
                            v_sb[h][:, mb * 65: mb * 65 + 65],
                            e3[:, hh, c0:],
                            start=(mb == 0),
                            stop=(mb == nm - 1),
                        )
                # ---- normalize pair's heads into otp ----
                # engine lanes are partition-hardwired, so a [1, 512]
                # denominator row would reciprocal on ONE lane (~3.3us).
                # Instead: copy the rows off PSUM, DMA-scatter both onto
                # 128 partitions, one cheap [128, 8] reciprocal, and
                # DMA-gather back to partition 64 for the bc matmul.
                den = [rcp.tile([65, 512], f32, tag="den", name=f"den{i}")
                       for i in range(2)]
                for hh in range(2):
                    nc.vector.tensor_copy(den[hh][64:65, :],
                                          o_ps[hh][64:65, :])
                d128 = rcp.tile([128, 8], f32, tag="d128")
                for hh in range(2):
                    nc.sync.dma_start(d128[:, 4 * hh:4 * hh + 4],
                                      den[hh][64:65, :])
                r128 = rcp.tile([128, 8], f32, tag="r128")
                nc.vector.reciprocal(r128[:], d128[:])
                rc = [rcp.tile([65, 512], f32, tag="rc", name=f"rc{i}")
                      for i in range(2)]
                for hh in range(2):
                    nc.sync.dma_start(rc[hh][64:65, :],
                                      r128[:, 4 * hh:4 * hh + 4])
                for hh in range(2):
                    bc = kvp.tile([64, 512], f32, tag="kv", name="bc")
                    nc.tensor.matmul(
                        bc[:], onesf32_sb[64:65, :], rc[hh][64:65, :]
                    )
                    bcs = bcsp.tile([64, 512], bf, tag="bcs")
                    nc.vector.tensor_copy(bcs[:], bc[:])
                    if hh == 0:
                        nc.vector.tensor_mul(
                            otp[pair][0:64, j * 512:(j + 1) * 512],
                            o_ps[0][0:64, :],
                            bcs[:],
                        )
                    else:
                        odd = oddp.tile([64, 512], bf, tag="odd")
                        nc.vector.tensor_mul(odd[:], o_ps[1][0:64, :], bcs[:])
                        nc.scalar.dma_start(
                            otp[pair][64:128, j * 512:(j + 1) * 512], odd[:]
                        )

            # ---- final projection for output rows of this n-block ----
            # (PE gap-filler while the next block's attention is ACT-bound)
            for sub in range(4):
                nbk = 4 * j + sub
                os_t = osp.tile([128, D], bf, tag="os")
                for cb in range(2):
                    fp = kvp.tile([128, 512], f32, tag="kv", name="fp")
                    for p2 in range(2):
                        nc.tensor.matmul(
                            fp[:],
                            otp[p2][:, nbk * 128:(nbk + 1) * 128],
                            wp_sb[:, p2 * D + cb * 512: p2 * D + (cb + 1) * 512],
                            start=(p2 == 0),
                            stop=(p2 == 1),
                        )
                    if sub % 2 == 0:
                        nc.scalar.copy(os_t[:, cb * 512:(cb + 1) * 512], fp[:])
                    else:
                        nc.vector.tensor_copy(
                            os_t[:, cb * 512:(cb + 1) * 512], fp[:]
                        )
                nc.scalar.dma_start(out_p[nbk * 128:(nbk + 1) * 128, :], os_t[:])

    nc.compile()
    return nc


def _get_program():
    if "nc" not in _CACHE:
        _CACHE["nc"] = _build_program()
    return _CACHE["nc"]


def _prep_in_maps(x, Wkqv, bkqv, Wproj, bproj):
    import ml_dtypes
    bf = ml_dtypes.bfloat16

    x = np.asarray(x, np.float32)
    Wkqv = np.asarray(Wkqv, np.float32)
    bkqv = np.asarray(bkqv, np.float32)
    Wproj = np.asarray(Wproj, np.float32)

    # de-interleave kqv columns: col 3d+0 -> k_d, 3d+1 -> q_d, 3d+2 -> v_d
    Wk = Wkqv[:, :, 0::3]  # [H, D, DK]
    Wq = Wkqv[:, :, 1::3]
    Wv = Wkqv[:, :, 2::3]
    bk = bkqv[:, 0::3]     # [H, DK]
    bq = bkqv[:, 1::3]
    bv = bkqv[:, 2::3]

    # one 128x128 tril block (m <= n within block), twice side by side
    mm = np.arange(128)[:, None]
    nn = np.arange(128)[None, :]
    tril = (mm <= nn).astype(np.float32)
    mask2 = np.concatenate([tril, tril], axis=1).astype(bf)
    ident = np.eye(128, dtype=np.float32).astype(bf)

    def wlayout(Wg):  # [4, D, DK] -> [128, CH*256] (chunk-major, pair cols)
        arr = Wg.reshape(2, 2, CH, 128, DK)          # [pair, hh, ch, p, f]
        return np.ascontiguousarray(
            arr.transpose(3, 2, 0, 1, 4).reshape(128, CH * 256).astype(bf)
        )

    group_maps = []
    for g in range(GROUPS):
        hs = slice(g * HPC, (g + 1) * HPC)
        # bias seed rows: [ktype(3)][pair(2)] x (hh*64 + d)
        biasrow = np.zeros((1, 768), np.float32)
        for ti, bsrc in enumerate((bk, bq, bv)):
            for pair in range(2):
                for hh in range(2):
                    h = g * HPC + 2 * pair + hh
                    col = ti * 256 + pair * 128 + hh * 64
                    biasrow[0, col:col + 64] = bsrc[h]
        wp_c = np.ascontiguousarray(
            Wproj[g * HPC * DK:(g + 1) * HPC * DK]
            .reshape(2, 128, D).transpose(1, 0, 2).reshape(128, 2 * D)
            .astype(bf)
        )
        group_maps.append({
            "wk": wlayout(Wk[hs]),
            "wq": wlayout(Wq[hs]),
            "wv": wlayout(Wv[hs]),
            "wp": wp_c,
            "biasrow": biasrow.astype(bf),
            "onesrow": np.ones((1, 512), bf),
            "mask2": mask2,
            "ident": ident,
            "ones2d": np.ones((128, 16), bf),
            "onesf32": np.ones((1, 64), np.float32),
        })

    xTs = [np.ascontiguousarray(x[b].T.astype(bf)) for b in range(B)]
    in_maps = []
    for c in range(NCORES):
        b, g = c // GROUPS, c % GROUPS
        m = dict(group_maps[g])
        m["xT"] = xTs[b]
        in_maps.append(m)
    return in_maps


def _run(inputs, trace=False):
    from concourse.bass_utils import run_bass_kernel_spmd

    nc = _get_program()
    in_maps = _prep_in_maps(
        inputs["x"], inputs["Wkqv"], inputs["bkqv"], inputs["Wproj"], inputs["bproj"]
    )
    res = run_bass_kernel_spmd(nc, in_maps, core_ids=list(range(NCORES)), trace=trace)
    bproj = np.asarray(inputs["bproj"], np.float32)
    out = np.empty((B, N, D), np.float32)
    for b in range(B):
        acc = res.results[b * GROUPS]["out_p"].astype(np.float32)
        for g in range(1, GROUPS):
            acc = acc + res.results[b * GROUPS + g]["out_p"].astype(np.float32)
        out[b] = acc + bproj[None, :]
    return out, res


def kernel(**inputs):
    return _run(inputs)[0]


# revision 29
# speedup vs baseline: 1.1489x; 1.1489x over previous
"""Causal self-attention (K/Q swapped variant) on 8 trn2 NeuronCores.

Sharding: core c = (b, g) with b = c // 4 (batch), g = c % 4 (head group of
4 heads).  Each core computes, for its batch and heads, the full attention
and a partial output projection (its heads' rows of Wproj); the host sums
the 4 bf16 partials per batch (fp32 accumulate) and adds bproj.

Per-core kernel (bf16 matmuls, fp32 PSUM accumulation):
  - x[b]^T arrives pre-transposed (bf16) from host as [D, N].
  - K^T, Q^T, V^T per head-pair: [128, 512] PSUM tiles (2 heads stacked on
    partitions) via W-stationary matmuls; k/q/v biases are folded in as a
    K=1 seed matmul (bias ⊗ ones) that starts each accumulation group.
  - V^T is PE-transposed into per-head [m, 65] tiles whose 65th column is
    1.0 (softmax row-sums fall out of the O matmul for free).
  - Scores for both heads of a pair go into one [128, 1024] PSUM quad
    (hh0 cols 0:512, hh1 cols 512:1024, different banks so the two K=64
    row-tiled matmuls run concurrently); ONE ACT exp per quad (scale=1/8),
    with the live-column trim applied exactly on diagonal quads.
  - Causal masking: one GPSIMD tensor-tensor multiply per diagonal quad
    over the two 128-wide on-diagonal blocks (3D access pattern); fully
    masked columns are never computed or read.
  - O_aug = V_aug^T . E accumulated over m-blocks: rows 0-63 unnormalized
    output^T, row 64 the softmax denominator.
  - normalize: reciprocal_approx_fast of row 64 straight off PSUM, PE
    outer-product broadcast to 64 partitions, one DVE multiply fusing the
    bf16 cast into otp.
  - partial out (bf16) = sum_h O_h^T.T @ Wproj[head rows] in PSUM, emitted
    per n-block so the projection fills PE gaps during the next block's
    attention.

PSUM budget (8 banks): scores quads 2x[128,1024] = 4, O accumulators
2x[65,512] = 2 (pairs processed serially), shared kqv/transpose/bc/proj
pool 2x[128,512] = 2.
"""

import os
import sys

if "/opt/trn_rl_repo" not in sys.path:
    sys.path.insert(0, "/opt/trn_rl_repo")

import numpy as np

B, N, D, H = 2, 2048, 1024, 16
DK = 64
NCORES = 8
GROUPS = 4          # head groups
HPC = H // GROUPS   # 4 heads per core
CH = D // 128       # 8 contraction chunks
NB = N // 512       # 4 n-blocks
MBS = N // 128      # 16 m-blocks

_CACHE = {}


def _build_program():
    import concourse.bacc as bacc
    import concourse.mybir as mybir
    from concourse.tile import TileContext
    from contextlib import ExitStack

    f32 = mybir.dt.float32
    bf = mybir.dt.bfloat16
    EXP = mybir.ActivationFunctionType.Exp

    nc = bacc.Bacc(
        "TRN2",
        target_bir_lowering=False,
        debug=False,
        enable_asserts=False,
        num_devices=NCORES,
    )

    xT = nc.dram_tensor("xT", [D, N], bf, kind="ExternalInput").ap()
    wk = nc.dram_tensor("wk", [128, CH * 256], bf, kind="ExternalInput").ap()
    wq = nc.dram_tensor("wq", [128, CH * 256], bf, kind="ExternalInput").ap()
    wv = nc.dram_tensor("wv", [128, CH * 256], bf, kind="ExternalInput").ap()
    wp = nc.dram_tensor("wp", [128, 2 * D], bf, kind="ExternalInput").ap()
    biasrow = nc.dram_tensor("biasrow", [1, 768], bf, kind="ExternalInput").ap()
    onesrow = nc.dram_tensor("onesrow", [1, 512], bf, kind="ExternalInput").ap()
    mask2 = nc.dram_tensor("mask2", [128, 256], bf, kind="ExternalInput").ap()
    ident = nc.dram_tensor("ident", [128, 128], bf, kind="ExternalInput").ap()
    ones2d = nc.dram_tensor("ones2d", [128, 16], bf, kind="ExternalInput").ap()
    onesf32 = nc.dram_tensor("onesf32", [1, 64], f32, kind="ExternalInput").ap()
    out_p = nc.dram_tensor("out_p", [N, D], bf, kind="ExternalOutput").ap()

    with TileContext(nc) as tc, ExitStack() as ctx:
        constp = ctx.enter_context(tc.tile_pool(name="const", bufs=1))
        storep = ctx.enter_context(tc.tile_pool(name="store", bufs=1))
        xtp = ctx.enter_context(tc.tile_pool(name="xt", bufs=16))
        vtp = ctx.enter_context(tc.tile_pool(name="vt", bufs=2))
        ep = ctx.enter_context(tc.tile_pool(name="e", bufs=6))
        rcp = ctx.enter_context(tc.tile_pool(name="rc", bufs=2))
        bcsp = ctx.enter_context(tc.tile_pool(name="bcs", bufs=3))
        oddp = ctx.enter_context(tc.tile_pool(name="odd", bufs=2))
        osp = ctx.enter_context(tc.tile_pool(name="os", bufs=3))
        sqp = ctx.enter_context(tc.tile_pool(name="sq", bufs=2, space="PSUM"))
        op_ = ctx.enter_context(tc.tile_pool(name="o", bufs=2, space="PSUM"))
        kvp = ctx.enter_context(tc.tile_pool(name="kv", bufs=2, space="PSUM"))

        # ---- constants / weights / x in SBUF ----
        # DMA traffic is split across the two HWDGE queues (sync, scalar);
        # 2-4KB per-partition lines keep packets big.  All of x loads up
        # front so kqv projections are always available as PE filler.
        wk_sb = constp.tile([128, CH * 256], bf, tag="wk")
        wq_sb = constp.tile([128, CH * 256], bf, tag="wq")
        wv_sb = constp.tile([128, CH * 256], bf, tag="wv")
        wp_sb = constp.tile([128, 2 * D], bf, tag="wp")
        biasrow_sb = constp.tile([1, 768], bf, tag="biasrow")
        onesrow_sb = constp.tile([1, 512], bf, tag="onesrow")
        mask2_sb = constp.tile([128, 256], bf, tag="mask2")
        ident_sb = constp.tile([128, 128], bf, tag="ident")
        ones2d_sb = constp.tile([128, 16], bf, tag="ones2d")
        # row 64 = ones; lives at partition 64 so the bc matmul's K=1
        # stationary and its rc moving operand share a base partition
        onesf32_sb = constp.tile([65, 64], f32, tag="onesf32")
        # rows 64-65: [1,0...;0,1...] selector for the per-pair bc matmul
        nc.sync.dma_start(biasrow_sb[:], biasrow[:, :])
        nc.sync.dma_start(onesrow_sb[:], onesrow[:, :])
        nc.sync.dma_start(ident_sb[:], ident[:, :])
        nc.sync.dma_start(mask2_sb[:], mask2[:, :])
        nc.sync.dma_start(ones2d_sb[:], ones2d[:, :])
        nc.sync.dma_start(onesf32_sb[64:65, :], onesf32[:, :])
        nc.sync.dma_start(wk_sb[:], wk[:, :])

        # x^T in two [128, 1024] halves per chunk, even chunks on the sync
        # queue, odd on the scalar queue
        xt = [[None] * CH for _ in range(2)]
        for half in range(2):
            for c in range(CH):
                t = xtp.tile([128, 1024], bf, tag="xt", name=f"xt{half}{c}")
                eng = nc.sync if c % 2 == 0 else nc.scalar
                eng.dma_start(
                    t[:],
                    xT[c * 128:(c + 1) * 128, half * 1024:(half + 1) * 1024],
                )
                xt[half][c] = t
            if half == 0:
                nc.scalar.dma_start(wq_sb[:], wq[:, :])
                nc.sync.dma_start(wv_sb[:], wv[:, :])
        nc.scalar.dma_start(wp_sb[:], wp[:, :])

        # ---- persistent activation storage ----
        kt = storep.tile([128, 2 * N], bf, tag="kt")    # [pairfeat, pair*N + n]
        qt = storep.tile([128, 2 * N], bf, tag="qt")
        v_sb = [storep.tile([128, MBS * 65], bf, tag=f"v{h}", name=f"v{h}")
                for h in range(HPC)]
        otp = [storep.tile([128, N], bf, tag=f"otp{p}", name=f"otp{p}")
               for p in range(2)]
        for h in range(HPC):
            nc.vector.tensor_copy(
                v_sb[h].rearrange("p (m c) -> p m c", c=65)[:, :, 64],
                ones2d_sb[:, 0:16],
            )

        mask3 = mask2_sb.rearrange("p (h c) -> p h c", h=2)

        for j in range(NB):
            nb = j
            xnb = [xt[nb // 2][c][:, (nb % 2) * 512:(nb % 2 + 1) * 512]
                   for c in range(CH)]

            # ---- K^T, Q^T, V^T projections for this n-block ----
            # bias folded in as a K=1 seed matmul starting each group
            for pair in range(2):
                for ti, (wsb, dst) in enumerate(((wk_sb, kt), (wq_sb, qt))):
                    ps = kvp.tile([128, 512], f32, tag="kv", name="kqv")
                    bcol = ti * 256 + pair * 128
                    nc.tensor.matmul(
                        ps[:], biasrow_sb[0:1, bcol:bcol + 128],
                        onesrow_sb[0:1, :], start=True, stop=False,
                    )
                    for c in range(CH):
                        nc.tensor.matmul(
                            ps[:],
                            wsb[:, c * 256 + pair * 128: c * 256 + (pair + 1) * 128],
                            xnb[c],
                            start=False,
                            stop=(c == CH - 1),
                        )
                    nc.vector.tensor_copy(
                        dst[:, pair * N + nb * 512: pair * N + (nb + 1) * 512],
                        ps[:],
                    )
                ps = kvp.tile([128, 512], f32, tag="kv", name="kqv")
                bcol = 2 * 256 + pair * 128
                nc.tensor.matmul(
                    ps[:], biasrow_sb[0:1, bcol:bcol + 128],
                    onesrow_sb[0:1, :], start=True, stop=False,
                )
                for c in range(CH):
                    nc.tensor.matmul(
                        ps[:],
                        wv_sb[:, c * 256 + pair * 128: c * 256 + (pair + 1) * 128],
                        xnb[c],
                        start=False,
                        stop=(c == CH - 1),
                    )
                vt = vtp.tile([128, 512], bf, tag="vt")
                nc.vector.tensor_copy(vt[:], ps[:])
                # transpose V^T -> per-head [m, 65] tiles
                for sub in range(4):
                    mb = nb * 4 + sub
                    pst = kvp.tile([128, 128], bf, tag="kv", name="pst")
                    nc.tensor.transpose(
                        pst[:], vt[:, sub * 128:(sub + 1) * 128], ident_sb[:]
                    )
                    nc.vector.tensor_copy(
                        v_sb[2 * pair][:, mb * 65: mb * 65 + 64], pst[:, 0:64]
                    )
                    nc.vector.tensor_copy(
                        v_sb[2 * pair + 1][:, mb * 65: mb * 65 + 64], pst[:, 64:128]
                    )

            # ---- attention for n-block j (needs m-blocks <= 4j+3) ----
            # pairs processed serially (o pool has 2 bufs = 1 pair)
            nm = 4 * j + 4
            for pair in range(2):
                o_ps = {}
                for hh in range(2):
                    o_ps[hh] = op_.tile([65, 512], f32, tag="o",
                                        name=f"o{j}{pair}{hh}")
                for mb in range(nm):
                    rdiag = mb - 4 * j
                    c0 = 128 * rdiag if rdiag > 0 else 0
                    # scores quad: both heads, different PSUM banks, the
                    # two K=64 matmuls run row-tiled concurrently
                    sq = sqp.tile([128, 1024], f32, tag="sq")
                    for hh in range(2):
                        base = hh * 64
                        nc.tensor.matmul(
                            sq[:, hh * 512 + c0: (hh + 1) * 512],
                            qt[base:base + 64,
                               pair * N + mb * 128: pair * N + (mb + 1) * 128],
                            kt[base:base + 64,
                               pair * N + j * 512 + c0: pair * N + (j + 1) * 512],
                        )
                    # ONE exp per quad, exact live-column trim
                    e = ep.tile([128, 1024], bf, tag="e")
                    sq3 = sq.rearrange("p (h c) -> p h c", h=2)
                    e3 = e.rearrange("p (h c) -> p h c", h=2)
                    nc.scalar.activation(e3[:, :, c0:], sq3[:, :, c0:], EXP,
                                         scale=0.125)
                    if rdiag >= 0:
                        # zero the two on-diagonal 128-blocks (GPSIMD; the
                        # fully-masked cols [0:c0) are never read)
                        blk = slice(rdiag * 128, (rdiag + 1) * 128)
                        nc.gpsimd.tensor_mul(
                            e3[:, :, blk], e3[:, :, blk], mask3[:, :, 0:128]
                        )
                    for hh in range(2):
                        h = 2 * pair + hh
                        nc.tensor.matmul(
                            o_ps[hh][:, c0:512],
                            v_sb[h][:, mb * 65: mb * 65 + 65],
                            e3[:, hh, c0:],
                            start=(mb == 0),
                            stop=(mb == nm - 1),
                        )
                # ---- normalize pair's heads into otp ----
                # engine lanes are partition-hardwired, so a [1, 512]
                # denominator row would reciprocal on ONE lane (~3.3us).
                # Instead: copy the rows off PSUM, DMA-scatter both onto
                # 128 partitions, one cheap [128, 8] reciprocal, and
                # DMA-gather back to partition 64 for the bc matmul.
                den = [rcp.tile([65, 512], f32, tag="den", name=f"den{i}")
                       for i in range(2)]
                for hh in range(2):
                    nc.vector.tensor_copy(den[hh][64:65, :],
                                          o_ps[hh][64:65, :])
                d128 = rcp.tile([128, 8], f32, tag="d128")
                for hh in range(2):
                    nc.sync.dma_start(d128[:, 4 * hh:4 * hh + 4],
                                      den[hh][64:65, :])
                r128 = rcp.tile([128, 8], f32, tag="r128")
                nc.vector.reciprocal(r128[:], d128[:])
                rc = [rcp.tile([65, 512], f32, tag="rc", name=f"rc{i}")
                      for i in range(2)]
                for hh in range(2):
                    nc.sync.dma_start(rc[hh][64:65, :],
                                      r128[:, 4 * hh:4 * hh + 4])
                for hh in range(2):
                    bc = kvp.tile([64, 512], f32, tag="kv", name="bc")
                    nc.tensor.matmul(
                        bc[:], onesf32_sb[64:65, :], rc[hh][64:65, :]
                    )
                    bcs = bcsp.tile([64, 512], bf, tag="bcs")
                    nc.vector.tensor_copy(bcs[:], bc[:])
                    if hh == 0:
                        nc.vector.tensor_mul(
                            otp[pair][0:64, j * 512:(j + 1) * 512],
                            o_ps[0][0:64, :],
                            bcs[:],
                        )
                    else:
                        odd = oddp.tile([64, 512], bf, tag="odd")
                        nc.vector.tensor_mul(odd[:], o_ps[1][0:64, :], bcs[:])
                        nc.sync.dma_start(
                            otp[pair][64:128, j * 512:(j + 1) * 512], odd[:]
                        )

            # ---- final projection for output rows of this n-block ----
            # (PE gap-filler while the next block's attention is ACT-bound)
            for sub in range(4):
                nbk = 4 * j + sub
                os_t = osp.tile([128, D], bf, tag="os")
                for cb in range(2):
                    fp = kvp.tile([128, 512], f32, tag="kv", name="fp")
                    for p2 in range(2):
                        nc.tensor.matmul(
                            fp[:],
                            otp[p2][:, nbk * 128:(nbk + 1) * 128],
                            wp_sb[:, p2 * D + cb * 512: p2 * D + (cb + 1) * 512],
                            start=(p2 == 0),
                            stop=(p2 == 1),
                        )
                    nc.vector.tensor_copy(os_t[:, cb * 512:(cb + 1) * 512], fp[:])
                nc.sync.dma_start(out_p[nbk * 128:(nbk + 1) * 128, :], os_t[:])

    nc.compile()
    return nc


def _get_program():
    if "nc" not in _CACHE:
        _CACHE["nc"] = _build_program()
    return _CACHE["nc"]


def _prep_in_maps(x, Wkqv, bkqv, Wproj, bproj):
    import ml_dtypes
    bf = ml_dtypes.bfloat16

    x = np.asarray(x, np.float32)
    Wkqv = np.asarray(Wkqv, np.float32)
    bkqv = np.asarray(bkqv, np.float32)
    Wproj = np.asarray(Wproj, np.float32)

    # de-interleave kqv columns: col 3d+0 -> k_d, 3d+1 -> q_d, 3d+2 -> v_d
    Wk = Wkqv[:, :, 0::3]  # [H, D, DK]
    Wq = Wkqv[:, :, 1::3]
    Wv = Wkqv[:, :, 2::3]
    bk = bkqv[:, 0::3]     # [H, DK]
    bq = bkqv[:, 1::3]
    bv = bkqv[:, 2::3]

    # one 128x128 tril block (m <= n within block), twice side by side
    mm = np.arange(128)[:, None]
    nn = np.arange(128)[None, :]
    tril = (mm <= nn).astype(np.float32)
    mask2 = np.concatenate([tril, tril], axis=1).astype(bf)
    ident = np.eye(128, dtype=np.float32).astype(bf)

    def wlayout(Wg):  # [4, D, DK] -> [128, CH*256] (chunk-major, pair cols)
        arr = Wg.reshape(2, 2, CH, 128, DK)          # [pair, hh, ch, p, f]
        return np.ascontiguousarray(
            arr.transpose(3, 2, 0, 1, 4).reshape(128, CH * 256).astype(bf)
        )

    group_maps = []
    for g in range(GROUPS):
        hs = slice(g * HPC, (g + 1) * HPC)
        # bias seed rows: [ktype(3)][pair(2)] x (hh*64 + d)
        biasrow = np.zeros((1, 768), np.float32)
        for ti, bsrc in enumerate((bk, bq, bv)):
            for pair in range(2):
                for hh in range(2):
                    h = g * HPC + 2 * pair + hh
                    col = ti * 256 + pair * 128 + hh * 64
                    biasrow[0, col:col + 64] = bsrc[h]
        wp_c = np.ascontiguousarray(
            Wproj[g * HPC * DK:(g + 1) * HPC * DK]
            .reshape(2, 128, D).transpose(1, 0, 2).reshape(128, 2 * D)
            .astype(bf)
        )
        group_maps.append({
            "wk": wlayout(Wk[hs]),
            "wq": wlayout(Wq[hs]),
            "wv": wlayout(Wv[hs]),
            "wp": wp_c,
            "biasrow": biasrow.astype(bf),
            "onesrow": np.ones((1, 512), bf),
            "mask2": mask2,
            "ident": ident,
            "ones2d": np.ones((128, 16), bf),
            "onesf32": np.ones((1, 64), np.float32),
        })

    xTs = [np.ascontiguousarray(x[b].T.astype(bf)) for b in range(B)]
    in_maps = []
    for c in range(NCORES):
        b, g = c // GROUPS, c % GROUPS
        m = dict(group_maps[g])
        m["xT"] = xTs[b]
        in_maps.append(m)
    return in_maps


def _run(inputs, trace=False):
    from concourse.bass_utils import run_bass_kernel_spmd

    nc = _get_program()
    in_maps = _prep_in_maps(
        inputs["x"], inputs["Wkqv"], inputs["bkqv"], inputs["Wproj"], inputs["bproj"]
    )
    res = run_bass_kernel_spmd(nc, in_maps, core_ids=list(range(NCORES)), trace=trace)
    bproj = np.asarray(inputs["bproj"], np.float32)
    out = np.empty((B, N, D), np.float32)
    for b in range(B):
        acc = res.results[b * GROUPS]["out_p"].astype(np.float32)
        for g in range(1, GROUPS):
            acc = acc + res.results[b * GROUPS + g]["out_p"].astype(np.float32)
        out[b] = acc + bproj[None, :]
    return out, res


def kernel(**inputs):
    return _run(inputs)[0]
